# revision 5
# baseline (speedup 1.0000x reference)
"""MemTransformerLM (Transformer-XL) forward pass on 8 TRN2 NeuronCores.

Sharding: core c handles batch b = c//2 and tensor-parallel half h = c%2
(heads 8h..8h+8 of 16; FFN inner columns 2048h..2048h+2048 of 4096).
Pairwise AllReduce (cores 2b, 2b+1) after the attention output projection and
after FFN. Vocab for the final logsumexp is split 16000 per core in the pair;
host combines per-tile (max, sumexp) partials and computes the NLL.

All matmuls run in bf16 with fp32 PSUM accumulation; the residual stream,
layernorm statistics, and softmax denominators stay fp32.

rel_shift trick: for unmasked positions (j <= i + mlen), Transformer-XL's
rel_shift satisfies shifted[i, j] = pre_flat[i*(klen-1) + j + (qlen-1)] where
pre_flat is the un-shifted [qlen, klen] score matrix viewed flat. We write pre
to DRAM contiguously and reload through a [row_stride=klen-1] access pattern;
masked positions read garbage that the mask then kills (only the j >= 512
half needs masking). Softmax skips max-subtraction: |scores*scale| < ~8 here,
exp is safe in fp32, and masked entries are exactly exp(-1e30*scale) = 0.
"""

import numpy as np
import ml_dtypes

import concourse.bass as bass
import concourse.mybir as mybir
import concourse.tile as tile
from concourse import bacc
from concourse.bass_utils import run_bass_kernel_spmd
from concourse.masks import make_identity

# Model dims (hardcoded per problem spec)
L = 6
D_MODEL = 1024
D_HEAD = 64
D_INNER = 4096
BSZ = 4
QLEN = 512
MLEN = 512
KLEN = MLEN + QLEN
VOCAB = 32000
SCALE = 1.0 / (D_HEAD ** 0.5)
EPS = 1e-5

NCORES = 8
NDH = 512          # nd per core (8 heads x 64)
DIH = 2048         # ffn inner per core
VSH = VOCAB // 2   # vocab per core (split across the pair)
VT = 400           # vocab tile width
NVT = VSH // VT    # 40

DT = mybir.dt.float32
BF = mybir.dt.bfloat16
F32 = np.float32
BF16 = ml_dtypes.bfloat16

PAIRS = [[0, 1], [2, 3], [4, 5], [6, 7]]

_CACHE: dict = {}


def _build():
    nc = bacc.Bacc("TRN2", target_bir_lowering=False, debug=False, num_devices=NCORES)

    # ---- I/O ----
    x0_in = nc.dram_tensor("x0", [QLEN, D_MODEL], DT, kind="ExternalInput")
    memT_in = nc.dram_tensor("memT", [L, D_MODEL, MLEN], BF, kind="ExternalInput")
    wq_in = nc.dram_tensor("wq", [L, D_MODEL, NDH], BF, kind="ExternalInput")
    wk_in = nc.dram_tensor("wk", [L, D_MODEL, NDH], BF, kind="ExternalInput")
    wv_in = nc.dram_tensor("wv", [L, D_MODEL, NDH], BF, kind="ExternalInput")
    rkT_in = nc.dram_tensor("rkT", [L, 4, 128, KLEN], BF, kind="ExternalInput")
    wo_in = nc.dram_tensor("wo", [L, NDH, D_MODEL], BF, kind="ExternalInput")
    w1_in = nc.dram_tensor("w1", [L, D_MODEL, DIH], BF, kind="ExternalInput")
    w2_in = nc.dram_tensor("w2", [L, DIH, D_MODEL], BF, kind="ExternalInput")
    b1_in = nc.dram_tensor("b1", [L, DIH], DT, kind="ExternalInput")
    b2_in = nc.dram_tensor("b2", [L, D_MODEL], DT, kind="ExternalInput")
    g1_in = nc.dram_tensor("g1", [L, D_MODEL], DT, kind="ExternalInput")
    bg1_in = nc.dram_tensor("bg1", [L, D_MODEL], DT, kind="ExternalInput")
    g2_in = nc.dram_tensor("g2", [L, D_MODEL], DT, kind="ExternalInput")
    bg2_in = nc.dram_tensor("bg2", [L, D_MODEL], DT, kind="ExternalInput")
    bw_in = nc.dram_tensor("bw", [NDH], DT, kind="ExternalInput")
    br_in = nc.dram_tensor("br", [NDH], DT, kind="ExternalInput")
    embT_in = nc.dram_tensor("embT", [D_MODEL, VSH], BF, kind="ExternalInput")

    xout = nc.dram_tensor("xout", [QLEN, D_MODEL], DT, kind="ExternalOutput")
    lmax_out = nc.dram_tensor("lmax", [128, 4, NVT], DT, kind="ExternalOutput")
    lsum_out = nc.dram_tensor("lsum", [128, 4, NVT], DT, kind="ExternalOutput")

    with tile.TileContext(nc) as tc:
        with (
            tc.tile_pool(name="const", bufs=1) as constp,
            tc.tile_pool(name="res", bufs=1) as resp,
            tc.tile_pool(name="wts", bufs=1) as wtp,
            tc.tile_pool(name="act", bufs=1) as actp,
            tc.tile_pool(name="xt2", bufs=2) as xtp,
            tc.tile_pool(name="ncc", bufs=2) as nccp,
            tc.tile_pool(name="arp", bufs=2) as arp,
            tc.tile_pool(name="tr", bufs=3) as trp,
            tc.tile_pool(name="pr2", bufs=2) as prp,
            tc.tile_pool(name="small", bufs=4) as smp,
            tc.tile_pool(name="ps_sc", bufs=2, space="PSUM") as psS,
            tc.tile_pool(name="ps_out", bufs=2, space="PSUM") as psO,
            tc.tile_pool(name="ps_proj", bufs=2, space="PSUM") as psP,
            tc.tile_pool(name="ps_tp", bufs=1, space="PSUM") as psT,
            tc.tile_pool(name="ps_pv", bufs=1, space="PSUM") as psV,
            tc.tile_pool(name="dram", bufs=2, space="DRAM") as dramp,
        ):
            ident = constp.tile([128, 128], BF)
            make_identity(nc, ident[:])
            identf = constp.tile([128, 128], DT)
            make_identity(nc, identf[:])
            bw_t = constp.tile([128, 4], DT)
            br_t = constp.tile([128, 4], DT)
            nc.sync.dma_start(bw_t[:], bw_in.rearrange("(c p) -> p c", p=128))
            nc.sync.dma_start(br_t[:], br_in.rearrange("(c p) -> p c", p=128))

            # residual stream, fp32, natural layout [part=q%128, qc, d]
            x_res = resp.tile([128, 4, D_MODEL], DT)
            nc.sync.dma_start(x_res[:], x0_in.rearrange("(c p) d -> p c d", p=128))
            lmax_sb = resp.tile([128, 4, NVT], DT)
            lsum_sb = resp.tile([128, 4, NVT], DT)

            def transpose_x():
                """PE-transpose x_res (fp32 in, bf16 out) into a fresh [128,8,QLEN] tile."""
                dest = xtp.tile([128, 8, QLEN], BF, tag="xt")
                for dc in range(8):
                    for qc in range(4):
                        tp = psT.tile([128, 128], DT, tag="tp")
                        nc.tensor.transpose(
                            tp[:], x_res[:, qc, 128 * dc : 128 * dc + 128], identf[:]
                        )
                        dst = dest[:, dc, 128 * qc : 128 * qc + 128]
                        if (dc + qc) % 2 == 0:
                            nc.scalar.copy(dst, tp[:])
                        else:
                            nc.vector.tensor_copy(dst, tp[:])
                return dest

            for l in range(L):
                # ---- weight loads (wq aliases wo's slot: disjoint lifetimes) ----
                wq_t = wtp.tile([128, 8, NDH], BF, tag="wqo")
                wk_t = wtp.tile([128, 8, NDH], BF, tag="wk")
                wv_t = wtp.tile([128, 8, NDH], BF, tag="wv")
                w1_t = wtp.tile([128, 8, DIH], BF, tag="wff")
                nc.sync.dma_start(wq_t[:], wq_in[l].rearrange("(c p) n -> p c n", p=128))
                nc.sync.dma_start(wk_t[:], wk_in[l].rearrange("(c p) n -> p c n", p=128))
                nc.sync.dma_start(wv_t[:], wv_in[l].rearrange("(c p) n -> p c n", p=128))
                nc.sync.dma_start(w1_t[:], w1_in[l].rearrange("(c p) n -> p c n", p=128))
                b1_t = wtp.tile([128, 16], DT, tag="bb")
                nc.sync.dma_start(b1_t[:], b1_in[l].rearrange("(c p) -> p c", p=128))

                memT_t = actp.tile([128, 8, MLEN], BF, tag="memT")
                nc.sync.dma_start(
                    memT_t[:], memT_in[l].rearrange("(c p) m -> p c m", p=128)
                )
                xT_t = transpose_x()

                # ---- attention, two heads (one ncc group) at a time ----
                pvT_all = actp.tile([128, 4, QLEN], BF, tag="pvT")

                def attn_head(ncc, hh, qbwT, qbrT, kT, vv, rkT):
                    base = 64 * hh
                    scr = dramp.tile([QLEN * KLEN + KLEN], BF, tag="bdsc")
                    scr2d = scr[: QLEN * KLEN].rearrange("(q k) -> q k", k=KLEN)
                    # pre = (q+br)^T-chunk @ rkT, written to DRAM
                    for qc in range(4):
                        for kh in range(2):
                            pre = psS.tile([128, 512], DT, tag="sc")
                            nc.tensor.matmul(
                                pre[:],
                                qbrT[base : base + 64, 128 * qc : 128 * qc + 128],
                                rkT[base : base + 64, 512 * kh : 512 * kh + 512],
                                start=True, stop=True,
                            )
                            pre_sb = trp.tile([128, 512], BF, tag="pre_sb")
                            nc.scalar.copy(pre_sb[:], pre[:])
                            nc.sync.dma_start(
                                scr2d[128 * qc : 128 * qc + 128,
                                      512 * kh : 512 * kh + 512],
                                pre_sb[:],
                            )
                    for qc in range(4):
                        attn_head_qc(ncc, hh, qc, qbwT, kT, vv, scr)

                def attn_head_qc(ncc, hh, qc, qbwT, kT, vv, scr):
                    base = 64 * hh
                    prob = prp.tile([128, KLEN], BF, tag="prob")
                    dens = smp.tile([128, 2], DT, tag="den")
                    for kh in range(2):
                        ac = psS.tile([128, 512], DT, tag="sc")
                        nc.tensor.matmul(
                            ac[:],
                            qbwT[base : base + 64, 128 * qc : 128 * qc + 128],
                            kT[base : base + 64, 512 * kh : 512 * kh + 512],
                            start=True, stop=True,
                        )
                        # shifted reload of pre rows [128*qc ..) cols [512*kh ..)
                        bd = trp.tile([128, 512], BF, tag="bd")
                        shifted = bass.AP(
                            scr.tensor,
                            scr.offset + (QLEN - 1)
                            + 128 * qc * (KLEN - 1) + 512 * kh,
                            [[KLEN - 1, 128], [1, 512]],
                        )
                        nc.sync.dma_start(bd[:], shifted)
                        if kh == 1:
                            # mask: keep j <= i + MLEN; i = 128*qc + p,
                            # j = 512 + jj  ->  iota = 128*qc + p - jj >= 0
                            nc.gpsimd.affine_select(
                                out=bd[:], in_=bd[:],
                                pattern=[[-1, 512]],
                                compare_op=mybir.AluOpType.is_ge,
                                fill=-1e30, base=128 * qc,
                                channel_multiplier=1,
                            )
                        # s = AC + BD, in place in PSUM
                        nc.vector.tensor_tensor(
                            ac[:], ac[:], bd[:], mybir.AluOpType.add
                        )
                        nc.scalar.activation(
                            prob[:, 512 * kh : 512 * kh + 512], ac[:],
                            mybir.ActivationFunctionType.Exp,
                            scale=SCALE, accum_out=dens[:, kh : kh + 1],
                        )
                    den = smp.tile([128, 1], DT, tag="dent")
                    nc.vector.tensor_tensor(
                        den[:], dens[:, 0:1], dens[:, 1:2], mybir.AluOpType.add
                    )
                    rec = smp.tile([128, 1], DT, tag="rec")
                    nc.vector.reciprocal(rec[:], den[:])
                    nc.vector.tensor_scalar_mul(prob[:], prob[:], rec[:])
                    pv = psV.tile([64, 128], DT, tag="pv")
                    for kc in range(8):
                        tp = psT.tile([128, 128], BF, tag="tp")
                        nc.tensor.transpose(
                            tp[:], prob[:, 128 * kc : 128 * kc + 128], ident[:]
                        )
                        ptsb = trp.tile([128, 128], BF, tag="ptsb")
                        if kc % 2 == 0:
                            nc.scalar.copy(ptsb[:], tp[:])
                        else:
                            nc.vector.tensor_copy(ptsb[:], tp[:])
                        nc.tensor.matmul(
                            pv[:], vv[:, kc, base : base + 64], ptsb[:],
                            start=(kc == 0), stop=(kc == 7),
                        )
                    nc.scalar.copy(
                        pvT_all[base : base + 64, ncc,
                                128 * qc : 128 * qc + 128],
                        pv[:],
                    )

                for ncc in range(4):
                    nsl = slice(128 * ncc, 128 * ncc + 128)
                    # q^T (+bw / +br) for this ncc chunk
                    qps = psP.tile([128, QLEN], DT, tag="proj")
                    for dc in range(8):
                        nc.tensor.matmul(
                            qps[:], wq_t[:, dc, nsl], xT_t[:, dc, :],
                            start=(dc == 0), stop=(dc == 7),
                        )
                    qbwT = nccp.tile([128, QLEN], BF, tag="qbw")
                    qbrT = nccp.tile([128, QLEN], BF, tag="qbr")
                    nc.scalar.add(qbwT[:], qps[:], bw_t[:, ncc : ncc + 1])
                    nc.scalar.add(qbrT[:], qps[:], br_t[:, ncc : ncc + 1])
                    # k^T for this ncc chunk
                    kT = nccp.tile([128, KLEN], BF, tag="kT")
                    for kh in range(2):
                        kps = psP.tile([128, QLEN], DT, tag="proj")
                        src = memT_t if kh == 0 else xT_t
                        for dc in range(8):
                            nc.tensor.matmul(
                                kps[:], wk_t[:, dc, nsl], src[:, dc, :],
                                start=(dc == 0), stop=(dc == 7),
                            )
                        nc.vector.tensor_copy(kT[:, 512 * kh : 512 * kh + 512], kps[:])
                    # v (natural) for this ncc chunk: [klen-part, kc, 2 heads]
                    vv = nccp.tile([128, 8, 128], BF, tag="vv")
                    for kc in range(8):
                        vps = psP.tile([128, QLEN], DT, tag="proj")
                        src = memT_t if kc < 4 else xT_t
                        ksl = slice(128 * (kc % 4), 128 * (kc % 4) + 128)
                        for dc in range(8):
                            nc.tensor.matmul(
                                vps[:, 0:128], src[:, dc, ksl], wv_t[:, dc, nsl],
                                start=(dc == 0), stop=(dc == 7),
                            )
                        if kc % 2 == 0:
                            nc.scalar.copy(vv[:, kc, :], vps[:, 0:128])
                        else:
                            nc.vector.tensor_copy(vv[:, kc, :], vps[:, 0:128])
                    # rk^T for this ncc chunk (host-computed rk = pos_emb @ Wr)
                    rkT = nccp.tile([128, KLEN], BF, tag="rkT")
                    nc.sync.dma_start(rkT[:], rkT_in[l, ncc])

                    for hh in range(2):
                        attn_head(ncc, hh, qbwT, qbrT, kT, vv, rkT)

                # ---- attention out projection + pairwise AllReduce (bf16) ----
                wo_t = wtp.tile([128, 4, D_MODEL], BF, tag="wqo")
                nc.sync.dma_start(wo_t[:], wo_in[l].rearrange("(c p) n -> p c n", p=128))
                ar_in = dramp.tile([QLEN, D_MODEL], BF, tag="arin")
                ar_out = dramp.tile([QLEN, D_MODEL], BF, tag="arout")
                for qc in range(4):
                    for kh in range(2):
                        ops = psO.tile([128, 512], DT, tag="out")
                        for ncc in range(4):
                            nc.tensor.matmul(
                                ops[:],
                                pvT_all[:, ncc, 128 * qc : 128 * qc + 128],
                                wo_t[:, ncc, 512 * kh : 512 * kh + 512],
                                start=(ncc == 0), stop=(ncc == 3),
                            )
                        asb = arp.tile([128, 512], BF, tag="ar_sb")
                        nc.vector.tensor_copy(asb[:], ops[:])
                        nc.sync.dma_start(
                            ar_in.rearrange("(c p) d -> p c d", p=128)[
                                :, qc, 512 * kh : 512 * kh + 512
                            ],
                            asb[:],
                        )
                nc.gpsimd.collective_compute(
                    "AllReduce", mybir.AluOpType.add,
                    replica_groups=PAIRS, ins=[ar_in.opt()], outs=[ar_out.opt()],
                )
                arr4 = ar_out.rearrange("(c p) d -> p c d", p=128)
                for qc in range(4):
                    arr = arp.tile([128, D_MODEL], BF, tag="arr")
                    nc.sync.dma_start(arr[:], arr4[:, qc, :])
                    nc.vector.tensor_tensor(
                        x_res[:, qc, :], x_res[:, qc, :], arr[:], mybir.AluOpType.add
                    )

                # layernorm params (bf16, broadcast to all partitions);
                # aliases the wq/wo slot (both dead by now)
                lnb = wtp.tile([128, 4, D_MODEL], BF, tag="wqo")
                for i, src in enumerate((g1_in, bg1_in, g2_in, bg2_in)):
                    lnrow = smp.tile([1, D_MODEL], BF, tag="lnrow")
                    nc.gpsimd.dma_start(lnrow[:], src[l : l + 1, :])
                    nc.gpsimd.partition_broadcast(lnb[:, i, :], lnrow[:])

                def layer_norm(goff):
                    for qc in range(4):
                        xr = x_res[:, qc, :]
                        ssum = smp.tile([128, 1], DT, tag="ssum")
                        nc.vector.tensor_reduce(
                            ssum[:], xr, mybir.AxisListType.X, mybir.AluOpType.add
                        )
                        mu = smp.tile([128, 1], DT, tag="mu")
                        nc.vector.tensor_scalar_mul(mu[:], ssum[:], 1.0 / D_MODEL)
                        nc.vector.tensor_scalar(
                            xr, xr, mu[:], None, mybir.AluOpType.subtract
                        )
                        sq = prp.tile([128, KLEN], BF, tag="prob")  # scratch
                        vsum = smp.tile([128, 1], DT, tag="vsum")
                        nc.scalar.activation(
                            sq[:, 0:D_MODEL], xr, mybir.ActivationFunctionType.Square,
                            accum_out=vsum[:],
                        )
                        std = smp.tile([128, 1], DT, tag="std")
                        nc.vector.tensor_scalar(
                            std[:], vsum[:], 1.0 / D_MODEL, EPS,
                            mybir.AluOpType.mult, mybir.AluOpType.add,
                        )
                        nc.scalar.sqrt(std[:], std[:])
                        rstd = smp.tile([128, 1], DT, tag="rstd")
                        nc.vector.reciprocal(rstd[:], std[:])
                        nc.vector.tensor_scalar_mul(xr, xr, rstd[:])
                        nc.vector.tensor_tensor(
                            xr, xr, lnb[:, 2 * goff, :], mybir.AluOpType.mult
                        )
                        nc.vector.tensor_tensor(
                            xr, xr, lnb[:, 2 * goff + 1, :], mybir.AluOpType.add
                        )

                layer_norm(0)

                # ---- FFN ----
                xfT = transpose_x()
                hT = actp.tile([128, 16, QLEN], BF, tag="hT")
                for ic in range(16):
                    ps = psP.tile([128, QLEN], DT, tag="proj")
                    for dc in range(8):
                        nc.tensor.matmul(
                            ps[:], w1_t[:, dc, 128 * ic : 128 * ic + 128],
                            xfT[:, dc, :],
                            start=(dc == 0), stop=(dc == 7),
                        )
                    nc.scalar.activation(
                        hT[:, ic, :], ps[:], mybir.ActivationFunctionType.Relu,
                        bias=b1_t[:, ic : ic + 1],
                    )
                w2_t = wtp.tile([128, 16, D_MODEL], BF, tag="wff")
                nc.sync.dma_start(w2_t[:], w2_in[l].rearrange("(c p) n -> p c n", p=128))
                ar_in2 = dramp.tile([QLEN, D_MODEL], BF, tag="arin")
                ar_out2 = dramp.tile([QLEN, D_MODEL], BF, tag="arout")
                for qc in range(4):
                    for kh in range(2):
                        ops = psO.tile([128, 512], DT, tag="out")
                        for ic in range(16):
                            nc.tensor.matmul(
                                ops[:],
                                hT[:, ic, 128 * qc : 128 * qc + 128],
                                w2_t[:, ic, 512 * kh : 512 * kh + 512],
                                start=(ic == 0), stop=(ic == 15),
                            )
                        asb = arp.tile([128, 512], BF, tag="ar_sb")
                        nc.vector.tensor_copy(asb[:], ops[:])
                        nc.sync.dma_start(
                            ar_in2.rearrange("(c p) d -> p c d", p=128)[
                                :, qc, 512 * kh : 512 * kh + 512
                            ],
                            asb[:],
                        )
                nc.gpsimd.collective_compute(
                    "AllReduce", mybir.AluOpType.add,
                    replica_groups=PAIRS, ins=[ar_in2.opt()], outs=[ar_out2.opt()],
                )
                # b2 broadcast (aliases b1's slot)
                b2b = wtp.tile([128, D_MODEL], BF, tag="bb")
                b2row = smp.tile([1, D_MODEL], BF, tag="b2row")
                nc.gpsimd.dma_start(b2row[:], b2_in[l : l + 1, :])
                nc.gpsimd.partition_broadcast(b2b[:], b2row[:])
                arr4b = ar_out2.rearrange("(c p) d -> p c d", p=128)
                for qc in range(4):
                    arr = arp.tile([128, D_MODEL], BF, tag="arr")
                    nc.sync.dma_start(arr[:], arr4b[:, qc, :])
                    nc.vector.tensor_tensor(
                        x_res[:, qc, :], x_res[:, qc, :], arr[:], mybir.AluOpType.add
                    )
                    nc.vector.tensor_tensor(
                        x_res[:, qc, :], x_res[:, qc, :], b2b[:], mybir.AluOpType.add
                    )
                layer_norm(1)

            # ---- final hidden out + unembed partials ----
            nc.sync.dma_start(xout.rearrange("(c p) d -> p c d", p=128), x_res[:])
            uf = transpose_x()
            embT_r = embT_in.rearrange("(c p) v -> p c v", p=128)
            for vt in range(NVT):
                # double-buffer embT tiles by ping-ponging two dead weight slots
                et = wtp.tile([128, 8, VT], BF, tag=("wk" if vt % 2 == 0 else "wv"))
                nc.sync.dma_start(et[:], embT_r[:, :, VT * vt : VT * vt + VT])
                for qc in range(4):
                    lps = psP.tile([128, QLEN], DT, tag="proj")
                    for dc in range(8):
                        nc.tensor.matmul(
                            lps[:, 0:VT],
                            uf[:, dc, 128 * qc : 128 * qc + 128],
                            et[:, dc, :],
                            start=(dc == 0), stop=(dc == 7),
                        )
                    nc.vector.tensor_reduce(
                        lmax_sb[:, qc, vt : vt + 1], lps[:, 0:VT],
                        mybir.AxisListType.X, mybir.AluOpType.max,
                    )
                    negm = smp.tile([128, 1], DT, tag="negm")
                    nc.vector.tensor_scalar_mul(
                        negm[:], lmax_sb[:, qc, vt : vt + 1], -1.0
                    )
                    lsc = trp.tile([128, VT], BF, tag="lsc")
                    nc.scalar.activation(
                        lsc[:], lps[:, 0:VT], mybir.ActivationFunctionType.Exp,
                        bias=negm[:], accum_out=lsum_sb[:, qc, vt : vt + 1],
                    )
            nc.sync.dma_start(lmax_out[:], lmax_sb[:])
            nc.sync.dma_start(lsum_out[:], lsum_sb[:])

    nc.compile()
    return nc


def _get_nc():
    if "nc" not in _CACHE:
        _CACHE["nc"] = _build()
    return _CACHE["nc"]


def _make_pos():
    pos_seq = np.arange(KLEN - 1, -1, -1, dtype=F32)
    inv_freq = 1.0 / (10000.0 ** (np.arange(0, D_MODEL, 2, dtype=F32) / D_MODEL))
    sin_inp = np.outer(pos_seq, inv_freq).astype(F32)
    return np.concatenate([np.sin(sin_inp), np.cos(sin_inp)], -1).astype(F32)


def _prep_inputs(data, memory, emb, Wq, Wkv, Wr, Wo, ffW1, ffb1, ffW2, ffb2,
                 ln1_g, ln1_b, ln2_g, ln2_b, bias_w, bias_r):
    pos = _make_pos()                                  # [KLEN, D_MODEL]
    rk = np.einsum("kd,ldn->lkn", pos, Wr.astype(F32))  # [L, KLEN, NDH*2]
    embT = np.ascontiguousarray(emb.T).astype(BF16)    # [D_MODEL, VOCAB]
    bwf = bias_w.reshape(-1).astype(F32)
    brf = bias_r.reshape(-1).astype(F32)
    in_maps = []
    for c in range(NCORES):
        b, h = c // 2, c % 2
        nds = slice(NDH * h, NDH * h + NDH)
        dis = slice(DIH * h, DIH * h + DIH)
        # rkT: [L, 4, 128, KLEN] = rk[:, :, nds].T chunked by 128 nd rows
        rkTh = np.ascontiguousarray(
            rk[:, :, nds].transpose(0, 2, 1).reshape(L, 4, 128, KLEN)
        ).astype(BF16)
        in_maps.append({
            "x0": np.ascontiguousarray(emb[np.asarray(data[b])]).astype(F32),
            "memT": np.ascontiguousarray(memory[:, b].transpose(0, 2, 1)).astype(BF16),
            "wq": np.ascontiguousarray(Wq[:, :, nds]).astype(BF16),
            "wk": np.ascontiguousarray(Wkv[:, :, nds]).astype(BF16),
            "wv": np.ascontiguousarray(
                Wkv[:, :, D_MODEL + NDH * h : D_MODEL + NDH * h + NDH]).astype(BF16),
            "rkT": rkTh,
            "wo": np.ascontiguousarray(Wo[:, nds, :]).astype(BF16),
            "w1": np.ascontiguousarray(ffW1[:, :, dis]).astype(BF16),
            "w2": np.ascontiguousarray(ffW2[:, dis, :]).astype(BF16),
            "b1": np.ascontiguousarray(ffb1[:, dis]).astype(F32),
            "b2": np.asarray(ffb2).astype(F32),
            "g1": np.asarray(ln1_g).astype(F32),
            "bg1": np.asarray(ln1_b).astype(F32),
            "g2": np.asarray(ln2_g).astype(F32),
            "bg2": np.asarray(ln2_b).astype(F32),
            "bw": np.ascontiguousarray(bwf[nds]),
            "br": np.ascontiguousarray(brf[nds]),
            "embT": np.ascontiguousarray(embT[:, VSH * h : VSH * h + VSH]),
        })
    return in_maps


def _combine(results, target, emb):
    nll = np.zeros((BSZ, QLEN), dtype=np.float64)
    for b in range(BSZ):
        r0, r1 = results[2 * b], results[2 * b + 1]
        lm = np.concatenate([r0["lmax"], r1["lmax"]], axis=-1).astype(np.float64)
        ls = np.concatenate([r0["lsum"], r1["lsum"]], axis=-1).astype(np.float64)
        M = lm.max(-1)                                   # [128, 4]
        Z = (ls * np.exp(lm - M[..., None])).sum(-1)     # [128, 4]
        logZ = (M + np.log(Z)).transpose(1, 0).reshape(QLEN)  # i = 128*qc + p
        xf = r0["xout"].astype(BF16).astype(np.float64)
        et = emb[np.asarray(target[b])].astype(BF16).astype(np.float64)
        tgt = (xf * et).sum(-1)
        nll[b] = logZ - tgt
    return nll.astype(F32).reshape(-1).reshape(QLEN, BSZ)


def kernel(**inputs):
    nc = _get_nc()
    data = np.asarray(inputs["data"])
    target = np.asarray(inputs["target"])
    emb = np.asarray(inputs["emb"], dtype=F32)
    in_maps = _prep_inputs(
        data, np.asarray(inputs["memory"], dtype=F32), emb,
        np.asarray(inputs["Wq"], dtype=F32), np.asarray(inputs["Wkv"], dtype=F32),
        np.asarray(inputs["Wr"], dtype=F32), np.asarray(inputs["Wo"], dtype=F32),
        np.asarray(inputs["ffW1"], dtype=F32), np.asarray(inputs["ffb1"], dtype=F32),
        np.asarray(inputs["ffW2"], dtype=F32), np.asarray(inputs["ffb2"], dtype=F32),
        np.asarray(inputs["ln1_g"], dtype=F32), np.asarray(inputs["ln1_b"], dtype=F32),
        np.asarray(inputs["ln2_g"], dtype=F32), np.asarray(inputs["ln2_b"], dtype=F32),
        np.asarray(inputs["bias_w"], dtype=F32), np.asarray(inputs["bias_r"], dtype=F32),
    )
    res = run_bass_kernel_spmd(nc, in_maps, core_ids=list(range(NCORES)))
    return _combine(res.results, target, emb)
